# revision 2
# baseline (speedup 1.0000x reference)
"""Trainium2 Bass kernel for nn_DynamicConv (per-pixel dynamic 5x5 conv, 8 heads).

Reference computation (per batch image b):
    f[i, j, :]  = sum_c x[b, c, i, j] * filt_w[c, :]          # (56,56,200)
    out[c, i, j] = sum_{kh,kw} xpad[c, i+kh, j+kw] * f[i, j, kh, kw, c//24]

Sharding: data-parallel over batch, but each core takes 2 images x one
28-column half of the width so that 112 of 128 SBUF partitions carry
(row, image) pairs: partition q = 2*row + img.  Compute-engine APs must
start at partition 0 (quad-aligned), so the five kh row shifts are
materialized as five separately-laid-out DRAM loads x_d0..x_d4
(x_dk[q, c, jp] = xpad[img, c, i+k, jp]); the kw shifts are free-dim
offsets.

VERSION=8 (fp16 pipeline; v4 fp32 kept for reference):
 - Everything 16-bit on chip: x loads, filt weights, generated filters,
   products, and the output store are float16; only PSUM accumulation
   (exact) is fp32.  DVE tensor ops hit the 2x_1p fast path (2-byte
   packed operands) and PE matmuls run 1 cycle/col.
 - PE: filter generation per output column j with the (96ch x 112px)
   channel-major x slice stationary against fw (96 x 200), fp16 in,
   fp32 PSUM, ACT evacuates to f_sb[q, kl, head, j] as fp16.
 - DVE: per (channel-half, kh) ONE fused multiply covering all 5 kw
   taps: in0 is an overlapping sliding-window AP on x_dk
   [q, kw, head, ch, j] (kw and j both stride-1 over the padded row),
   in1 broadcasts f across the 24 channels of each head (step-0 AP).
 - PE: sums the 25 taps into PSUM via fp16 identity matmuls
   (6 x 448-col chunks per half), ACT evacuates to an fp16 out tile.
 - fp16 rounding of x/fw/f/products gives ~2e-3 scale-relative absmax
   error (harness gate is 2e-2); fp32 v4 measured 1.3e-4.
"""

import numpy as np

import bass_rust
import concourse.bass as bass
import concourse.bacc as bacc
import concourse.mybir as mybir
import concourse.tile as tile
from concourse.bass_utils import run_bass_kernel_spmd

B, C, H, W = 8, 192, 56, 56
K, HEADS = 5, 8
CG = C // HEADS            # 24 channels per head
FCOLS = K * K * HEADS      # 200 filter-gen outputs per pixel
WH = 28                    # columns per core (half width)
JP = WH + 4                # padded columns held in SBUF
P_O = 2 * H                # 112 partitions carrying (row, img) pairs
JCH = 7                    # filter-gen j-columns per chunk
N_CORES = 8

F32 = mybir.dt.float32
F16 = mybir.dt.float16

VERSION = 8

F32R = mybir.dt.float32r
HHEADS = HEADS // 2        # heads per channel-half
NCH = 6                    # PSUM chunks per half
CHF = 96 * WH // NCH       # 448 fp32 per chunk = 16 channels x 28 cols


def build_nc(version=None):
    if version is None:
        version = VERSION
    if version >= 8:
        return build_nc_v8()
    return build_nc_v4(version)


def build_nc_v8():
    nc = bacc.Bacc(None)

    xd_in = [
        nc.dram_tensor(f"x_d{k}", [P_O, C, JP], F16, kind="ExternalInput")
        for k in range(K)
    ]
    xg_in = nc.dram_tensor("x_gen", [96, 2, WH, P_O], F16, kind="ExternalInput")
    fw_in = nc.dram_tensor("fw_pk", [96, 2, FCOLS], F16, kind="ExternalInput")
    id_in = nc.dram_tensor("identh", [P_O, P_O], F16, kind="ExternalInput")
    out_d = nc.dram_tensor("out_sbl", [P_O, C, WH], F16, kind="ExternalOutput")

    with tile.TileContext(nc) as tc:
        with (
            tc.tile_pool(name="big", bufs=1) as big,
            tc.tile_pool(name="pr", bufs=2) as pr,
            tc.tile_pool(name="ps_f", bufs=2, space="PSUM") as ps_f,
            tc.tile_pool(name="ps_a", bufs=NCH, space="PSUM") as ps_a,
        ):
            fw_sb = big.tile([96, 2, FCOLS], F16)
            xg = big.tile([96, 2, WH, P_O], F16)
            xd = [
                big.tile([P_O, C, JP], F16, tag=f"xd{k}", name=f"xd{k}")
                for k in range(K)
            ]
            f_sb = big.tile([P_O, K * K, HEADS, WH], F16)
            acc = big.tile([P_O, C, WH], F16)
            ident = big.tile([P_O, P_O], F16)

            # Filter-gen operands first in the DMA queue so the PE can
            # start while the five big x_dk loads stream in behind.
            nc.sync.dma_start(fw_sb[:], fw_in[:])
            nc.sync.dma_start(xg[:], xg_in[:])
            nc.sync.dma_start(ident[:], id_in[:])
            for k in range(K):
                nc.sync.dma_start(xd[k][:], xd_in[k][:])

            # ---- filter generation: f[q, kl, h, j] = sum_c x[c, q, j] * fw[c, kl*8+h]
            for j in range(WH):
                fps = ps_f.tile([P_O, K * K, HEADS], F32, tag="fps")
                for ck in range(2):
                    nc.tensor.matmul(
                        fps[:],
                        xg[:, ck, j, :],       # (96 ch, 112 px) stationary
                        fw_sb[:, ck, :],       # (96 ch, 200)
                        start=(ck == 0),
                        stop=(ck == 1),
                    )
                nc.scalar.copy(f_sb[:, :, :, j], fps[:])

            # ---- conv: per half, DVE computes one fused 5-kw product tile
            # per kh; PE accumulates all 25 taps into PSUM.
            for hh in range(2):
                c0 = hh * 96
                accps = [
                    ps_a.tile([P_O, CHF], F32, tag="accps", name=f"accps{hh}_{b}")
                    for b in range(NCH)
                ]
                for kh in range(K):
                    # x window view [q, kw, h, g, j]: kw and j overlap on
                    # the stride-1 padded row axis.
                    xv = xd[kh][:, c0 : c0 + 96, 0:WH].copy()
                    pdim = xv.ap[0]
                    xv.ap = bass_rust.VecI64Pair(
                        [list(pdim), [1, K], [CG * JP, HHEADS], [JP, CG], [1, WH]]
                    )
                    fv = (
                        f_sb[:, 5 * kh : 5 * kh + 5, hh * HHEADS : (hh + 1) * HHEADS, :]
                        .unsqueeze(3)
                        .broadcast_to([P_O, K, HHEADS, CG, WH])
                    )
                    prod = pr.tile(
                        [P_O, K, HHEADS, CG, WH], F16, tag="prod",
                        name=f"prod{hh}_{kh}",
                    )
                    nc.vector.tensor_mul(prod[:], xv, fv)

                    pm = prod[:].rearrange("p kw h g j -> p kw (h g j)")
                    for kw in range(K):
                        for b in range(NCH):
                            nc.tensor.matmul(
                                accps[b][:],
                                ident[:],
                                pm[:, kw, b * CHF : (b + 1) * CHF],
                                start=(kh == 0 and kw == 0),
                                stop=(kh == K - 1 and kw == K - 1),
                            )
                for b in range(NCH):
                    nc.scalar.copy(
                        acc[:, c0 + b * 16 : c0 + (b + 1) * 16, :],
                        accps[b][:].rearrange("p (c j) -> p c j", j=WH),
                    )

            nc.sync.dma_start(out_d[:], acc[:])

    return nc


def build_nc_v4(version=4):
    nc = bacc.Bacc(None)

    xd_in = [
        nc.dram_tensor(f"x_d{k}", [P_O, C, JP], F32, kind="ExternalInput")
        for k in range(K)
    ]
    xg_in = nc.dram_tensor("x_gen", [96, 2, WH, P_O], F32, kind="ExternalInput")
    fw_in = nc.dram_tensor("fw_pk", [96, 2, FCOLS], F32, kind="ExternalInput")
    id_in = nc.dram_tensor("ident", [P_O, P_O], F32R, kind="ExternalInput")
    out_d = nc.dram_tensor("out_sbl", [P_O, C, WH], F32, kind="ExternalOutput")

    with tile.TileContext(nc) as tc:
        with (
            tc.tile_pool(name="big", bufs=1) as big,
            tc.tile_pool(name="sh", bufs=2) as sh,
            tc.tile_pool(name="ps_f", bufs=2, space="PSUM") as ps_f,
            tc.tile_pool(name="ps_a", bufs=NCH, space="PSUM") as ps_a,
        ):
            xd = [
                big.tile([P_O, C, JP], F32, tag=f"xd{k}", name=f"xd{k}")
                for k in range(K)
            ]
            fw_sb = big.tile([96, 2, FCOLS], F32)
            f_sb = big.tile([P_O, K * K, HEADS, WH], F32)
            acc = big.tile([P_O, C, WH], F32)
            ident = big.tile([P_O, P_O], F32R)

            nc.sync.dma_start(ident[:], id_in[:])
            for k in range(K):
                nc.sync.dma_start(xd[k][:], xd_in[k][:])
            nc.sync.dma_start(fw_sb[:], fw_in[:])

            for jc in range(WH // JCH):
                xg = sh.tile([96, 2, JCH, P_O], F32, tag="xgprod")
                nc.sync.dma_start(xg[:], xg_in[:, :, jc * JCH : (jc + 1) * JCH, :])
                for jl in range(JCH):
                    j = jc * JCH + jl
                    fps = ps_f.tile([P_O, K * K, HEADS], F32, tag="fps")
                    for ck in range(2):
                        nc.tensor.matmul(
                            fps[:],
                            xg[:, ck, jl, :],
                            fw_sb[:, ck, :],
                            start=(ck == 0),
                            stop=(ck == 1),
                        )
                    nc.scalar.copy(f_sb[:, :, :, j], fps[:])

            for hh in range(2):
                c0 = hh * 96
                accps = [
                    ps_a.tile([P_O, CHF], F32, tag="accps", name=f"accps{hh}_{b}")
                    for b in range(NCH)
                ]
                for kl in range(K * K):
                    kh, kw = divmod(kl, K)
                    xin = xd[kh][:, c0 : c0 + 96, kw : kw + WH]
                    xin4 = xin.rearrange("p (h g) j -> p h g j", h=HHEADS)
                    fbc = (
                        f_sb[:, kl, hh * HHEADS : (hh + 1) * HHEADS, :]
                        .unsqueeze(2)
                        .broadcast_to([P_O, HHEADS, CG, WH])
                    )
                    prod = sh.tile(
                        [P_O, 96, WH], F32R, tag="xgprod", name=f"prod{hh}_{kl}",
                    )
                    p4 = prod[:].rearrange("p (h g) j -> p h g j", h=HHEADS)
                    nc.vector.tensor_mul(p4, xin4, fbc)
                    pflat = prod[:].rearrange("p c j -> p (c j)")
                    for b in range(NCH):
                        nc.tensor.matmul(
                            accps[b][:],
                            ident[:],
                            pflat[:, b * CHF : (b + 1) * CHF],
                            start=(kl == 0),
                            stop=(kl == K * K - 1),
                        )
                for b in range(NCH):
                    nc.scalar.copy(
                        acc[:, c0 + b * 16 : c0 + (b + 1) * 16, :],
                        accps[b][:].rearrange("p (c j) -> p c j", j=WH),
                    )

            nc.sync.dma_start(out_d[:], acc[:])

    return nc


def shard_inputs(x, filt_w, version=None):
    """Split full inputs into the 8 per-core input maps."""
    if version is None:
        version = VERSION
    dt = np.float16 if version >= 8 else np.float32
    x = np.ascontiguousarray(np.asarray(x, dtype=np.float32))
    fw = np.ascontiguousarray(np.asarray(filt_w, dtype=np.float32))
    fw_pk = np.ascontiguousarray(
        fw.reshape(2, 96, FCOLS).transpose(1, 0, 2)
    ).astype(dt)

    in_maps = []
    for core in range(N_CORES):
        pair, jh = divmod(core, 2)
        xs = x[2 * pair : 2 * pair + 2]           # (2, C, 56, 56)
        xpad = np.zeros((2, C, H + 4, JP), np.float32)
        lo = jh * WH - 2                           # global col of jp=0
        s_lo, s_hi = max(lo, 0), min(lo + JP, W)
        xpad[:, :, 2 : 2 + H, s_lo - lo : s_lo - lo + (s_hi - s_lo)] = xs[
            :, :, :, s_lo:s_hi
        ]
        m = {"fw_pk": fw_pk}
        if version >= 8:
            m["identh"] = np.eye(P_O, dtype=np.float16)
        else:
            m["ident"] = np.eye(P_O, dtype=np.float32)
        for k in range(K):
            # x_dk[2*i+img, c, jp] = xpad[img, c, i+k, jp]
            m[f"x_d{k}"] = np.ascontiguousarray(
                xpad[:, :, k : k + H, :].transpose(2, 0, 1, 3).reshape(P_O, C, JP)
            ).astype(dt)
        # channel-major copy for filter-gen: x_gen[c96, ck, j, 2*i+img]
        xs_half = xs[:, :, :, jh * WH : (jh + 1) * WH]  # (2, C, 56, 28)
        xg = xs_half.transpose(1, 3, 2, 0).reshape(C, WH, P_O)
        m["x_gen"] = np.ascontiguousarray(
            xg.reshape(2, 96, WH, P_O).transpose(1, 0, 2, 3)
        ).astype(dt)
        in_maps.append(m)
    return in_maps


def unshard_output(results):
    """Reassemble the 8 per-core outputs into the full (B, C, H, W) tensor."""
    out = np.empty((B, C, H, W), np.float32)
    for core in range(N_CORES):
        pair, jh = divmod(core, 2)
        arr = np.asarray(results[core]["out_sbl"]).astype(np.float32)
        arr = arr.reshape(H, 2, C, WH)
        # arr[i, img, c, j] = out[2*pair+img, c, i, jh*28+j]
        out[2 * pair : 2 * pair + 2, :, :, jh * WH : (jh + 1) * WH] = arr.transpose(
            1, 2, 0, 3
        )
    return out


_NC_CACHE = {}


def _get_nc(version=None):
    if version is None:
        version = VERSION
    if version not in _NC_CACHE:
        nc = build_nc(version)
        if not nc.is_finalized():
            nc.finalize()
        _NC_CACHE[version] = nc
    return _NC_CACHE[version]


def run(inputs, trace=False, version=None, **kwargs):
    """Run on the 8 NeuronCores; returns BassKernelResults."""
    in_maps = shard_inputs(inputs["x"], inputs["filt_w"], version=version)
    nc = _get_nc(version)
    return run_bass_kernel_spmd(
        nc, in_maps, core_ids=list(range(N_CORES)), trace=trace, **kwargs
    )


def kernel(x, filt_w):
    res = run({"x": x, "filt_w": filt_w})
    return unshard_output(res.results)


# revision 8
# speedup vs baseline: 1.9267x; 1.9267x over previous
"""Trainium2 Bass kernel for nn_DynamicConv (per-pixel dynamic 5x5 conv, 8 heads).

Reference computation (per batch image b):
    f[i, j, :]  = sum_c x[b, c, i, j] * filt_w[c, :]          # (56,56,200)
    out[c, i, j] = sum_{kh,kw} xpad[c, i+kh, j+kw] * f[i, j, kh, kw, c//24]

Sharding: data-parallel over batch, but each core takes 2 images x one
28-column half of the width so that 112 of 128 SBUF partitions carry
(row, image) pairs: partition q = 2*row + img.  Compute-engine APs must
start at partition 0 (quad-aligned), so the five kh row shifts are
materialized as five separately-laid-out DRAM loads x_d0..x_d4
(x_dk[q, c, jp] = xpad[img, c, i+k, jp]); the kw shifts are free-dim
offsets.

VERSION=8 (fp16 pipeline; v4 fp32 kept for reference):
 - Everything 16-bit on chip: x loads, filt weights, generated filters,
   products, and the output store are float16; only PSUM accumulation
   (exact) is fp32.  DVE tensor ops hit the 2x_1p fast path (2-byte
   packed operands) and PE matmuls run 1 cycle/col.
 - PE: filter generation per output column j with the (96ch x 112px)
   channel-major x slice stationary against fw (96 x 200), fp16 in,
   fp32 PSUM, ACT evacuates to f_sb[q, kl, head, j] as fp16.
 - DVE: per (channel-half, kh) ONE fused multiply covering all 5 kw
   taps: in0 is an overlapping sliding-window AP on x_dk
   [q, kw, head, ch, j] (kw and j both stride-1 over the padded row),
   in1 broadcasts f across the 24 channels of each head (step-0 AP).
 - PE: sums the 25 taps into PSUM via fp16 identity matmuls
   (6 x 448-col chunks per half), ACT evacuates to an fp16 out tile.
 - fp16 rounding of x/fw/f/products gives ~2e-3 scale-relative absmax
   error (harness gate is 2e-2); fp32 v4 measured 1.3e-4.
"""

import numpy as np

import bass_rust
import concourse.bass as bass
import concourse.bacc as bacc
import concourse.mybir as mybir
import concourse.tile as tile
from concourse.bass_utils import run_bass_kernel_spmd

B, C, H, W = 8, 192, 56, 56
K, HEADS = 5, 8
CG = C // HEADS            # 24 channels per head
FCOLS = K * K * HEADS      # 200 filter-gen outputs per pixel
WH = 28                    # columns per core (half width)
JP = WH + 4                # padded columns held in SBUF
P_O = 2 * H                # 112 partitions carrying (row, img) pairs
JCH = 7                    # filter-gen j-columns per chunk
N_CORES = 8

F32 = mybir.dt.float32
F16 = mybir.dt.float16

VERSION = 8

F32R = mybir.dt.float32r
HHEADS = HEADS // 2        # heads per channel-half
NCH = 6                    # PSUM chunks per half
CHF = 96 * WH // NCH       # 448 fp32 per chunk = 16 channels x 28 cols


def build_nc(version=None):
    if version is None:
        version = VERSION
    if version >= 8:
        return build_nc_v8()
    return build_nc_v4(version)


def build_nc_v8():
    nc = bacc.Bacc(None)

    xd_in = [
        nc.dram_tensor(f"x_d{k}", [P_O, C, JP], F16, kind="ExternalInput")
        for k in range(K)
    ]
    # Stationary operands carry 128 columns (output partitions) so the
    # compiler's Fast Weight Load kicks in (requires NumWeights==128 and a
    # non-fp32 dtype); the extra columns are zero and the extra PSUM
    # partitions are never read.
    xg_in = nc.dram_tensor("x_gen", [96, 2, WH, 128], F16, kind="ExternalInput")
    fw_in = nc.dram_tensor("fw_pk", [96, 2, FCOLS], F16, kind="ExternalInput")
    id_in = nc.dram_tensor("identh", [P_O, 128], F16, kind="ExternalInput")
    out_d = nc.dram_tensor("out_sbl", [P_O, C, WH], F16, kind="ExternalOutput")

    with tile.TileContext(nc) as tc:
        with (
            tc.tile_pool(name="big", bufs=1) as big,
            tc.tile_pool(name="pr", bufs=2) as pr,
            tc.tile_pool(name="ps_f", bufs=2, space="PSUM") as ps_f,
            tc.tile_pool(name="ps_a", bufs=NCH, space="PSUM") as ps_a,
        ):
            fw_sb = big.tile([96, 2, FCOLS], F16)
            xg = big.tile([96, 2, WH, 128], F16)
            xd = [
                big.tile([P_O, C, JP], F16, tag=f"xd{k}", name=f"xd{k}")
                for k in range(K)
            ]
            f_sb = big.tile([P_O, K * K, HEADS, WH], F16)
            acc = big.tile([P_O, C, WH], F16)
            ident = big.tile([P_O, 128], F16)

            # Filter-gen operands first in the DMA queue so the PE can
            # start while the five big x_dk loads stream in behind.
            nc.sync.dma_start(fw_sb[:], fw_in[:])
            nc.sync.dma_start(xg[:], xg_in[:])
            nc.sync.dma_start(ident[:], id_in[:])
            for k in range(K):
                nc.sync.dma_start(xd[k][:], xd_in[k][:])

            # ---- filter generation: f[q, kl, h, j] = sum_c x[c, q, j] * fw[c, kl*8+h]
            for j in range(WH):
                fps = ps_f.tile([128, K * K, HEADS], F32, tag="fps")
                for ck in range(2):
                    nc.tensor.matmul(
                        fps[:],
                        xg[:, ck, j, :],       # (96 ch, 128 px) stationary
                        fw_sb[:, ck, :],       # (96 ch, 200)
                        start=(ck == 0),
                        stop=(ck == 1),
                    )
                nc.scalar.copy(f_sb[:, :, :, j], fps[:P_O])

            # ---- conv: per half, DVE computes one fused 5-kw product tile
            # per kh; PE accumulates all 25 taps into PSUM.
            for hh in range(2):
                c0 = hh * 96
                accps = [
                    ps_a.tile([128, CHF], F32, tag="accps", name=f"accps{hh}_{b}")
                    for b in range(NCH)
                ]
                for kl in range(K * K):
                    kh, kw = divmod(kl, K)
                    # ISA limit: TensorTensor free APs are at most 3-D, and
                    # the g-broadcast of f needs its own step-0 dim, so the
                    # biggest product op is one (h, g, j) tap per half.
                    xin = xd[kh][:, c0 : c0 + 96, kw : kw + WH]
                    xin4 = xin.rearrange("p (h g) j -> p h g j", h=HHEADS)
                    fv = (
                        f_sb[:, kl, hh * HHEADS : (hh + 1) * HHEADS, :]
                        .unsqueeze(2)
                        .broadcast_to([P_O, HHEADS, CG, WH])
                    )
                    prod = pr.tile(
                        [P_O, HHEADS, CG, WH], F16, tag="prod",
                        name=f"prod{hh}_{kl}",
                    )
                    nc.vector.tensor_mul(prod[:], xin4, fv)

                    pm = prod[:].rearrange("p h g j -> p (h g j)")
                    for b in range(NCH):
                        nc.tensor.matmul(
                            accps[b][:],
                            ident[:],
                            pm[:, b * CHF : (b + 1) * CHF],
                            start=(kl == 0),
                            stop=(kl == K * K - 1),
                        )
                for b in range(NCH):
                    nc.scalar.copy(
                        acc[:, c0 + b * 16 : c0 + (b + 1) * 16, :],
                        accps[b][:P_O].rearrange("p (c j) -> p c j", j=WH),
                    )

            nc.sync.dma_start(out_d[:], acc[:])

    return nc


def build_nc_v4(version=4):
    nc = bacc.Bacc(None)

    xd_in = [
        nc.dram_tensor(f"x_d{k}", [P_O, C, JP], F32, kind="ExternalInput")
        for k in range(K)
    ]
    xg_in = nc.dram_tensor("x_gen", [96, 2, WH, P_O], F32, kind="ExternalInput")
    fw_in = nc.dram_tensor("fw_pk", [96, 2, FCOLS], F32, kind="ExternalInput")
    id_in = nc.dram_tensor("ident", [P_O, P_O], F32R, kind="ExternalInput")
    out_d = nc.dram_tensor("out_sbl", [P_O, C, WH], F32, kind="ExternalOutput")

    with tile.TileContext(nc) as tc:
        with (
            tc.tile_pool(name="big", bufs=1) as big,
            tc.tile_pool(name="sh", bufs=2) as sh,
            tc.tile_pool(name="ps_f", bufs=2, space="PSUM") as ps_f,
            tc.tile_pool(name="ps_a", bufs=NCH, space="PSUM") as ps_a,
        ):
            xd = [
                big.tile([P_O, C, JP], F32, tag=f"xd{k}", name=f"xd{k}")
                for k in range(K)
            ]
            fw_sb = big.tile([96, 2, FCOLS], F32)
            f_sb = big.tile([P_O, K * K, HEADS, WH], F32)
            acc = big.tile([P_O, C, WH], F32)
            ident = big.tile([P_O, P_O], F32R)

            nc.sync.dma_start(ident[:], id_in[:])
            for k in range(K):
                nc.sync.dma_start(xd[k][:], xd_in[k][:])
            nc.sync.dma_start(fw_sb[:], fw_in[:])

            for jc in range(WH // JCH):
                xg = sh.tile([96, 2, JCH, P_O], F32, tag="xgprod")
                nc.sync.dma_start(xg[:], xg_in[:, :, jc * JCH : (jc + 1) * JCH, :])
                for jl in range(JCH):
                    j = jc * JCH + jl
                    fps = ps_f.tile([P_O, K * K, HEADS], F32, tag="fps")
                    for ck in range(2):
                        nc.tensor.matmul(
                            fps[:],
                            xg[:, ck, jl, :],
                            fw_sb[:, ck, :],
                            start=(ck == 0),
                            stop=(ck == 1),
                        )
                    nc.scalar.copy(f_sb[:, :, :, j], fps[:])

            for hh in range(2):
                c0 = hh * 96
                accps = [
                    ps_a.tile([P_O, CHF], F32, tag="accps", name=f"accps{hh}_{b}")
                    for b in range(NCH)
                ]
                for kl in range(K * K):
                    kh, kw = divmod(kl, K)
                    xin = xd[kh][:, c0 : c0 + 96, kw : kw + WH]
                    xin4 = xin.rearrange("p (h g) j -> p h g j", h=HHEADS)
                    fbc = (
                        f_sb[:, kl, hh * HHEADS : (hh + 1) * HHEADS, :]
                        .unsqueeze(2)
                        .broadcast_to([P_O, HHEADS, CG, WH])
                    )
                    prod = sh.tile(
                        [P_O, 96, WH], F32R, tag="xgprod", name=f"prod{hh}_{kl}",
                    )
                    p4 = prod[:].rearrange("p (h g) j -> p h g j", h=HHEADS)
                    nc.vector.tensor_mul(p4, xin4, fbc)
                    pflat = prod[:].rearrange("p c j -> p (c j)")
                    for b in range(NCH):
                        nc.tensor.matmul(
                            accps[b][:],
                            ident[:],
                            pflat[:, b * CHF : (b + 1) * CHF],
                            start=(kl == 0),
                            stop=(kl == K * K - 1),
                        )
                for b in range(NCH):
                    nc.scalar.copy(
                        acc[:, c0 + b * 16 : c0 + (b + 1) * 16, :],
                        accps[b][:].rearrange("p (c j) -> p c j", j=WH),
                    )

            nc.sync.dma_start(out_d[:], acc[:])

    return nc


def shard_inputs(x, filt_w, version=None):
    """Split full inputs into the 8 per-core input maps."""
    if version is None:
        version = VERSION
    dt = np.float16 if version >= 8 else np.float32
    x = np.ascontiguousarray(np.asarray(x, dtype=np.float32))
    fw = np.ascontiguousarray(np.asarray(filt_w, dtype=np.float32))
    fw_pk = np.ascontiguousarray(
        fw.reshape(2, 96, FCOLS).transpose(1, 0, 2)
    ).astype(dt)

    in_maps = []
    for core in range(N_CORES):
        pair, jh = divmod(core, 2)
        xs = x[2 * pair : 2 * pair + 2]           # (2, C, 56, 56)
        xpad = np.zeros((2, C, H + 4, JP), np.float32)
        lo = jh * WH - 2                           # global col of jp=0
        s_lo, s_hi = max(lo, 0), min(lo + JP, W)
        xpad[:, :, 2 : 2 + H, s_lo - lo : s_lo - lo + (s_hi - s_lo)] = xs[
            :, :, :, s_lo:s_hi
        ]
        m = {"fw_pk": fw_pk}
        if version >= 8:
            m["identh"] = np.eye(P_O, 128, dtype=np.float16)
        else:
            m["ident"] = np.eye(P_O, dtype=np.float32)
        for k in range(K):
            # x_dk[2*i+img, c, jp] = xpad[img, c, i+k, jp]
            m[f"x_d{k}"] = np.ascontiguousarray(
                xpad[:, :, k : k + H, :].transpose(2, 0, 1, 3).reshape(P_O, C, JP)
            ).astype(dt)
        # channel-major copy for filter-gen: x_gen[c96, ck, j, 2*i+img]
        xs_half = xs[:, :, :, jh * WH : (jh + 1) * WH]  # (2, C, 56, 28)
        xg = xs_half.transpose(1, 3, 2, 0).reshape(C, WH, P_O)
        xg = xg.reshape(2, 96, WH, P_O).transpose(1, 0, 2, 3)
        if version >= 8:
            # pad the px (stationary-column) axis to 128 for FWL
            xgp = np.zeros((96, 2, WH, 128), np.float32)
            xgp[:, :, :, :P_O] = xg
            xg = xgp
        m["x_gen"] = np.ascontiguousarray(xg).astype(dt)
        in_maps.append(m)
    return in_maps


def unshard_output(results):
    """Reassemble the 8 per-core outputs into the full (B, C, H, W) tensor."""
    out = np.empty((B, C, H, W), np.float32)
    for core in range(N_CORES):
        pair, jh = divmod(core, 2)
        arr = np.asarray(results[core]["out_sbl"]).astype(np.float32)
        arr = arr.reshape(H, 2, C, WH)
        # arr[i, img, c, j] = out[2*pair+img, c, i, jh*28+j]
        out[2 * pair : 2 * pair + 2, :, :, jh * WH : (jh + 1) * WH] = arr.transpose(
            1, 2, 0, 3
        )
    return out


_NC_CACHE = {}


def _get_nc(version=None):
    if version is None:
        version = VERSION
    if version not in _NC_CACHE:
        nc = build_nc(version)
        if not nc.is_finalized():
            nc.finalize()
        _NC_CACHE[version] = nc
    return _NC_CACHE[version]


def run(inputs, trace=False, version=None, **kwargs):
    """Run on the 8 NeuronCores; returns BassKernelResults."""
    in_maps = shard_inputs(inputs["x"], inputs["filt_w"], version=version)
    nc = _get_nc(version)
    return run_bass_kernel_spmd(
        nc, in_maps, core_ids=list(range(N_CORES)), trace=trace, **kwargs
    )


def kernel(x, filt_w):
    res = run({"x": x, "filt_w": filt_w})
    return unshard_output(res.results)


# revision 11
# speedup vs baseline: 2.0076x; 1.0420x over previous
"""Trainium2 Bass kernel for nn_DynamicConv (per-pixel dynamic 5x5 conv, 8 heads).

Reference computation (per batch image b):
    f[i, j, :]  = sum_c x[b, c, i, j] * filt_w[c, :]          # (56,56,200)
    out[c, i, j] = sum_{kh,kw} xpad[c, i+kh, j+kw] * f[i, j, kh, kw, c//24]

Sharding: data-parallel over batch, but each core takes 2 images x one
28-column half of the width so that 112 of 128 SBUF partitions carry
(row, image) pairs: partition q = 2*row + img.  Compute-engine APs must
start at partition 0 (quad-aligned), so the five kh row shifts are
materialized as five separately-laid-out DRAM loads x_d0..x_d4
(x_dk[q, c, jp] = xpad[img, c, i+k, jp]); the kw shifts are free-dim
offsets.

VERSION=8 (fp16 pipeline; v4 fp32 kept for reference):
 - Everything 16-bit on chip: x loads, filt weights, generated filters,
   products, and the output store are float16; only PSUM accumulation
   (exact) is fp32.  DVE tensor ops hit the 2x_1p fast path (2-byte
   packed operands) and PE matmuls run 1 cycle/col.
 - PE: filter generation per output column j with the (96ch x 112px)
   channel-major x slice stationary against fw (96 x 200), fp16 in,
   fp32 PSUM, ACT evacuates to f_sb[q, kl, head, j] as fp16.
 - DVE: per (channel-half, kh) ONE fused multiply covering all 5 kw
   taps: in0 is an overlapping sliding-window AP on x_dk
   [q, kw, head, ch, j] (kw and j both stride-1 over the padded row),
   in1 broadcasts f across the 24 channels of each head (step-0 AP).
 - PE: sums the 25 taps into PSUM via fp16 identity matmuls
   (6 x 448-col chunks per half), ACT evacuates to an fp16 out tile.
 - fp16 rounding of x/fw/f/products gives ~2e-3 scale-relative absmax
   error (harness gate is 2e-2); fp32 v4 measured 1.3e-4.
"""

import numpy as np

import bass_rust
import concourse.bass as bass
import concourse.bacc as bacc
import concourse.mybir as mybir
import concourse.tile as tile
from concourse.bass_utils import run_bass_kernel_spmd

B, C, H, W = 8, 192, 56, 56
K, HEADS = 5, 8
CG = C // HEADS            # 24 channels per head
FCOLS = K * K * HEADS      # 200 filter-gen outputs per pixel
WH = 28                    # columns per core (half width)
JP = WH + 4                # padded columns held in SBUF
P_O = 2 * H                # 112 partitions carrying (row, img) pairs
JCH = 7                    # filter-gen j-columns per chunk
N_CORES = 8

F32 = mybir.dt.float32
F16 = mybir.dt.float16

VERSION = 8

F32R = mybir.dt.float32r
HHEADS = HEADS // 2        # heads per channel-half
NCH = 6                    # PSUM chunks per half
CHF = 96 * WH // NCH       # 448 fp32 per chunk = 16 channels x 28 cols


def build_nc(version=None):
    if version is None:
        version = VERSION
    if version >= 8:
        return build_nc_v8()
    return build_nc_v4(version)


def build_nc_v8():
    nc = bacc.Bacc(None)

    xd_in = [
        nc.dram_tensor(f"x_d{k}", [P_O, C, JP], F16, kind="ExternalInput")
        for k in range(K)
    ]
    # Stationary operands carry 128 columns (output partitions) so the
    # compiler's Fast Weight Load kicks in (requires NumWeights==128 and a
    # non-fp32 dtype); the extra columns are zero and the extra PSUM
    # partitions are never read.
    xg_in = nc.dram_tensor("x_gen", [96, 2, WH, 128], F16, kind="ExternalInput")
    fw_in = nc.dram_tensor("fw_pk", [96, 2, FCOLS], F16, kind="ExternalInput")
    id_in = nc.dram_tensor("identh", [P_O, 128], F16, kind="ExternalInput")
    out_d = nc.dram_tensor("out_sbl", [P_O, C, WH], F16, kind="ExternalOutput")

    with tile.TileContext(nc) as tc:
        with (
            tc.tile_pool(name="big", bufs=1) as big,
            tc.tile_pool(name="pr", bufs=3) as pr,
            tc.tile_pool(name="ps_f", bufs=2, space="PSUM") as ps_f,
            tc.tile_pool(name="ps_a", bufs=NCH, space="PSUM") as ps_a,
        ):
            fw_sb = big.tile([96, 2, FCOLS], F16)
            xg = big.tile([96, 2, WH, 128], F16)
            xd = [
                big.tile([P_O, C, JP], F16, tag=f"xd{k}", name=f"xd{k}")
                for k in range(K)
            ]
            f_sb = big.tile([P_O, K * K, HEADS, WH], F16)
            acc = big.tile([P_O, C, WH], F16)
            ident = big.tile([P_O, 128], F16)

            # SDMA engines round-robin between logical queues at packet
            # granularity, so concurrent big loads steal bandwidth from the
            # critical-path x_gen load.  Chain the five x_dk loads behind it
            # (and each other) so each transfer runs at full bandwidth in
            # need order: xg -> xd0 -> xd1 -> ... -> xd4.
            nc.sync.dma_start(fw_sb[:], fw_in[:])
            nc.sync.dma_start(ident[:], id_in[:])
            i_prev = nc.sync.dma_start(xg[:], xg_in[:])
            tc.chain_iter_dep("dma_order", i_prev.ins)
            for k in range(K):
                i_k = nc.sync.dma_start(xd[k][:], xd_in[k][:])
                tc.chain_iter_dep("dma_order", i_k.ins)

            # ---- filter generation: f[q, kl, h, j] = sum_c x[c, q, j] * fw[c, kl*8+h]
            # PSUM evacuation runs on the DVE (idle here, and ~2.5x faster
            # per copy than ACT, whose SBUF access latency dominated v8).
            for j in range(WH):
                fps = ps_f.tile([128, K * K, HEADS], F32, tag="fps")
                for ck in range(2):
                    nc.tensor.matmul(
                        fps[:],
                        xg[:, ck, j, :],       # (96 ch, 128 px) stationary
                        fw_sb[:, ck, :],       # (96 ch, 200)
                        start=(ck == 0),
                        stop=(ck == 1),
                    )
                nc.vector.tensor_copy(f_sb[:, :, :, j], fps[:P_O])

            # ---- conv: per half, DVE computes one fused 5-kw product tile
            # per kh; PE accumulates all 25 taps into PSUM.
            for hh in range(2):
                c0 = hh * 96
                accps = [
                    ps_a.tile([128, CHF], F32, tag="accps", name=f"accps{hh}_{b}")
                    for b in range(NCH)
                ]
                for kl in range(K * K):
                    kh, kw = divmod(kl, K)
                    # ISA limit: TensorTensor free APs are at most 3-D, and
                    # the g-broadcast of f needs its own step-0 dim, so the
                    # biggest product op is one (h, g, j) tap per half.
                    xin = xd[kh][:, c0 : c0 + 96, kw : kw + WH]
                    xin4 = xin.rearrange("p (h g) j -> p h g j", h=HHEADS)
                    fv = (
                        f_sb[:, kl, hh * HHEADS : (hh + 1) * HHEADS, :]
                        .unsqueeze(2)
                        .broadcast_to([P_O, HHEADS, CG, WH])
                    )
                    prod = pr.tile(
                        [P_O, HHEADS, CG, WH], F16, tag="prod",
                        name=f"prod{hh}_{kl}",
                    )
                    nc.vector.tensor_mul(prod[:], xin4, fv)

                    pm = prod[:].rearrange("p h g j -> p (h g j)")
                    for b in range(NCH):
                        nc.tensor.matmul(
                            accps[b][:],
                            ident[:],
                            pm[:, b * CHF : (b + 1) * CHF],
                            start=(kl == 0),
                            stop=(kl == K * K - 1),
                        )
                # PSUM evacuation + store per 16-channel chunk so the output
                # DMA overlaps the remaining compute.  The final half splits
                # evacuation between ACT and the (now idle) DVE to shorten
                # the tail.
                for b in range(NCH):
                    dst = acc[:, c0 + b * 16 : c0 + (b + 1) * 16, :]
                    src = accps[b][:P_O].rearrange("p (c j) -> p c j", j=WH)
                    if hh == 1 and b >= NCH // 2:
                        nc.vector.tensor_copy(dst, src)
                    else:
                        nc.scalar.copy(dst, src)
                    nc.sync.dma_start(
                        out_d[:, c0 + b * 16 : c0 + (b + 1) * 16, :], dst
                    )

    return nc


def build_nc_v4(version=4):
    nc = bacc.Bacc(None)

    xd_in = [
        nc.dram_tensor(f"x_d{k}", [P_O, C, JP], F32, kind="ExternalInput")
        for k in range(K)
    ]
    xg_in = nc.dram_tensor("x_gen", [96, 2, WH, P_O], F32, kind="ExternalInput")
    fw_in = nc.dram_tensor("fw_pk", [96, 2, FCOLS], F32, kind="ExternalInput")
    id_in = nc.dram_tensor("ident", [P_O, P_O], F32R, kind="ExternalInput")
    out_d = nc.dram_tensor("out_sbl", [P_O, C, WH], F32, kind="ExternalOutput")

    with tile.TileContext(nc) as tc:
        with (
            tc.tile_pool(name="big", bufs=1) as big,
            tc.tile_pool(name="sh", bufs=2) as sh,
            tc.tile_pool(name="ps_f", bufs=2, space="PSUM") as ps_f,
            tc.tile_pool(name="ps_a", bufs=NCH, space="PSUM") as ps_a,
        ):
            xd = [
                big.tile([P_O, C, JP], F32, tag=f"xd{k}", name=f"xd{k}")
                for k in range(K)
            ]
            fw_sb = big.tile([96, 2, FCOLS], F32)
            f_sb = big.tile([P_O, K * K, HEADS, WH], F32)
            acc = big.tile([P_O, C, WH], F32)
            ident = big.tile([P_O, P_O], F32R)

            nc.sync.dma_start(ident[:], id_in[:])
            for k in range(K):
                nc.sync.dma_start(xd[k][:], xd_in[k][:])
            nc.sync.dma_start(fw_sb[:], fw_in[:])

            for jc in range(WH // JCH):
                xg = sh.tile([96, 2, JCH, P_O], F32, tag="xgprod")
                nc.sync.dma_start(xg[:], xg_in[:, :, jc * JCH : (jc + 1) * JCH, :])
                for jl in range(JCH):
                    j = jc * JCH + jl
                    fps = ps_f.tile([P_O, K * K, HEADS], F32, tag="fps")
                    for ck in range(2):
                        nc.tensor.matmul(
                            fps[:],
                            xg[:, ck, jl, :],
                            fw_sb[:, ck, :],
                            start=(ck == 0),
                            stop=(ck == 1),
                        )
                    nc.scalar.copy(f_sb[:, :, :, j], fps[:])

            for hh in range(2):
                c0 = hh * 96
                accps = [
                    ps_a.tile([P_O, CHF], F32, tag="accps", name=f"accps{hh}_{b}")
                    for b in range(NCH)
                ]
                for kl in range(K * K):
                    kh, kw = divmod(kl, K)
                    xin = xd[kh][:, c0 : c0 + 96, kw : kw + WH]
                    xin4 = xin.rearrange("p (h g) j -> p h g j", h=HHEADS)
                    fbc = (
                        f_sb[:, kl, hh * HHEADS : (hh + 1) * HHEADS, :]
                        .unsqueeze(2)
                        .broadcast_to([P_O, HHEADS, CG, WH])
                    )
                    prod = sh.tile(
                        [P_O, 96, WH], F32R, tag="xgprod", name=f"prod{hh}_{kl}",
                    )
                    p4 = prod[:].rearrange("p (h g) j -> p h g j", h=HHEADS)
                    nc.vector.tensor_mul(p4, xin4, fbc)
                    pflat = prod[:].rearrange("p c j -> p (c j)")
                    for b in range(NCH):
                        nc.tensor.matmul(
                            accps[b][:],
                            ident[:],
                            pflat[:, b * CHF : (b + 1) * CHF],
                            start=(kl == 0),
                            stop=(kl == K * K - 1),
                        )
                for b in range(NCH):
                    nc.scalar.copy(
                        acc[:, c0 + b * 16 : c0 + (b + 1) * 16, :],
                        accps[b][:].rearrange("p (c j) -> p c j", j=WH),
                    )

            nc.sync.dma_start(out_d[:], acc[:])

    return nc


def shard_inputs(x, filt_w, version=None):
    """Split full inputs into the 8 per-core input maps."""
    if version is None:
        version = VERSION
    dt = np.float16 if version >= 8 else np.float32
    x = np.ascontiguousarray(np.asarray(x, dtype=np.float32))
    fw = np.ascontiguousarray(np.asarray(filt_w, dtype=np.float32))
    fw_pk = np.ascontiguousarray(
        fw.reshape(2, 96, FCOLS).transpose(1, 0, 2)
    ).astype(dt)

    in_maps = []
    for core in range(N_CORES):
        pair, jh = divmod(core, 2)
        xs = x[2 * pair : 2 * pair + 2]           # (2, C, 56, 56)
        xpad = np.zeros((2, C, H + 4, JP), np.float32)
        lo = jh * WH - 2                           # global col of jp=0
        s_lo, s_hi = max(lo, 0), min(lo + JP, W)
        xpad[:, :, 2 : 2 + H, s_lo - lo : s_lo - lo + (s_hi - s_lo)] = xs[
            :, :, :, s_lo:s_hi
        ]
        m = {"fw_pk": fw_pk}
        if version >= 8:
            m["identh"] = np.eye(P_O, 128, dtype=np.float16)
        else:
            m["ident"] = np.eye(P_O, dtype=np.float32)
        for k in range(K):
            # x_dk[2*i+img, c, jp] = xpad[img, c, i+k, jp]
            m[f"x_d{k}"] = np.ascontiguousarray(
                xpad[:, :, k : k + H, :].transpose(2, 0, 1, 3).reshape(P_O, C, JP)
            ).astype(dt)
        # channel-major copy for filter-gen: x_gen[c96, ck, j, 2*i+img]
        xs_half = xs[:, :, :, jh * WH : (jh + 1) * WH]  # (2, C, 56, 28)
        xg = xs_half.transpose(1, 3, 2, 0).reshape(C, WH, P_O)
        xg = xg.reshape(2, 96, WH, P_O).transpose(1, 0, 2, 3)
        if version >= 8:
            # pad the px (stationary-column) axis to 128 for FWL
            xgp = np.zeros((96, 2, WH, 128), np.float32)
            xgp[:, :, :, :P_O] = xg
            xg = xgp
        m["x_gen"] = np.ascontiguousarray(xg).astype(dt)
        in_maps.append(m)
    return in_maps


def unshard_output(results):
    """Reassemble the 8 per-core outputs into the full (B, C, H, W) tensor."""
    out = np.empty((B, C, H, W), np.float32)
    for core in range(N_CORES):
        pair, jh = divmod(core, 2)
        arr = np.asarray(results[core]["out_sbl"]).astype(np.float32)
        arr = arr.reshape(H, 2, C, WH)
        # arr[i, img, c, j] = out[2*pair+img, c, i, jh*28+j]
        out[2 * pair : 2 * pair + 2, :, :, jh * WH : (jh + 1) * WH] = arr.transpose(
            1, 2, 0, 3
        )
    return out


_NC_CACHE = {}


def _get_nc(version=None):
    if version is None:
        version = VERSION
    if version not in _NC_CACHE:
        nc = build_nc(version)
        if not nc.is_finalized():
            nc.finalize()
        _NC_CACHE[version] = nc
    return _NC_CACHE[version]


def run(inputs, trace=False, version=None, **kwargs):
    """Run on the 8 NeuronCores; returns BassKernelResults."""
    in_maps = shard_inputs(inputs["x"], inputs["filt_w"], version=version)
    nc = _get_nc(version)
    return run_bass_kernel_spmd(
        nc, in_maps, core_ids=list(range(N_CORES)), trace=trace, **kwargs
    )


def kernel(x, filt_w):
    res = run({"x": x, "filt_w": filt_w})
    return unshard_output(res.results)
